# revision 1
# baseline (speedup 1.0000x reference)
"""Trainium2 Bass kernel for nn_CMSWrite (scatter_memory).

Computation (see reference):
  - tiny MLP -> z[l], gate g[l] (thresholded), value v[l] (512), key k[l] (128)
  - scores = K @ k / sqrt(128); alpha = softmax over all 65536 slots per level
  - M_new = (1-decay)*M + g*alpha (x) v ; K_new = (1-decay)*K + g*alpha (x) k
  - out = concat(M_new, K_new, axis=-1)   [3, 65536, 640]

Sharding: slots (N=65536) split across 8 NeuronCores (8192 each). The tiny
MLP is replicated on every core. Softmax normalizer is computed with a
per-core partial sum of exp(score) followed by a 3-float AllReduce (scores
are O(1) for this problem so no max-subtraction is needed; exp cannot
overflow fp32).

Per-core phases:
  0. MLP on device (PE matmuls + DVE/ACT elementwise, all on partition 0 rows)
  1. Load K shard resident in SBUF, scores via DVE fused mul+reduce,
     exp via ACT (+partial sums), cross-partition sum via PE, AllReduce.
  2. Stream M tiles; PE rank-1 outer products (exp row (x) v') into PSUM;
     one fused DVE op per tile: out = (in * keep) + psum;  DMA results out.
"""

import math
import numpy as np

L = 3
N = 65536
DLVL = 512
DK = 128
DZ = 128
NCORES = 8
S = N // NCORES          # 8192 slots per core
T = S // 128             # 64 slot-tiles of 128
SUB = 8                  # 128-slot sub-tiles per streamed chunk
CH_SL = SUB * 128        # 1024 slots per chunk
NCH = S // CH_SL         # 8 chunks per level
# Slot permutation: within a chunk, partition p / sub-tile t holds slot
# c*1024 + p*8 + t. Used identically by phase-1 (K scores), the exp-row
# bounce, phase-2 streaming, and the output write, so it cancels out —
# while giving per-partition contiguous DMA runs of 8 rows.
EPS = 1e-5
THRESH = 0.1
SCALE = 1.0 / math.sqrt(DK)

_STATE = {}
SKIP_CC = False   # debug: replace AllReduce with local copy (wrong numerics)


def _build_bass():
    import concourse.bacc as bacc
    import concourse.tile as tile
    import concourse.mybir as mybir
    from concourse.masks import make_identity

    f32 = mybir.dt.float32
    bf16 = mybir.dt.bfloat16
    A = mybir.AluOpType
    AF = mybir.ActivationFunctionType
    AX = mybir.AxisListType

    nc = bacc.Bacc("TRN2", target_bir_lowering=False, debug=False,
                   num_devices=NCORES)

    # M and K pre-concatenated on host along the last axis: [..., 0:512]=M,
    # [..., 512:640]=K. One streaming input and one fully-contiguous output.
    MKp = nc.dram_tensor("MKp", [L, S, DLVL + DK], f32,
                         kind="ExternalInput").ap()
    # Contiguous K-only copy for the phase-1 score pass (4KB DMA runs).
    Kp = nc.dram_tensor("Kp", [L, S, DK], f32, kind="ExternalInput").ap()
    xcatT = nc.dram_tensor("xcatT", [L, 128, 14], f32, kind="ExternalInput").ap()
    wevT = nc.dram_tensor("wevT", [L, 1792, 128], f32, kind="ExternalInput").ap()
    wvalT = nc.dram_tensor("wvalT", [L, 128, DLVL], f32, kind="ExternalInput").ap()
    wkeyT = nc.dram_tensor("wkeyT", [L, 128, DK], f32, kind="ExternalInput").ap()
    bev_r = nc.dram_tensor("bev_r", [1, L * DZ], f32, kind="ExternalInput").ap()
    lng_r = nc.dram_tensor("lng_r", [1, L * DZ], f32, kind="ExternalInput").ap()
    lnb_r = nc.dram_tensor("lnb_r", [1, L * DZ], f32, kind="ExternalInput").ap()
    wg_r = nc.dram_tensor("wg_r", [1, L * DZ], f32, kind="ExternalInput").ap()
    bg_r = nc.dram_tensor("bg_r", [1, L], f32, kind="ExternalInput").ap()
    bval_r = nc.dram_tensor("bval_r", [1, L * DLVL], f32, kind="ExternalInput").ap()
    bkey_r = nc.dram_tensor("bkey_r", [1, L * DK], f32, kind="ExternalInput").ap()
    dec_r = nc.dram_tensor("dec_r", [1, L], f32, kind="ExternalInput").ap()

    out = nc.dram_tensor("out", [L, S, DLVL + DK], f32, kind="ExternalOutput").ap()

    with tile.TileContext(nc) as tc:
        with (
            tc.tile_pool(name="constp", bufs=1) as constp,
            tc.tile_pool(name="wp", bufs=2) as wp,
            tc.tile_pool(name="sm", bufs=1) as sm,
            tc.tile_pool(name="zrp", bufs=3) as zrp,
            tc.tile_pool(name="junkp", bufs=2) as junkp,
            tc.tile_pool(name="strm", bufs=1) as strm,
            tc.tile_pool(name="pmisc", bufs=3, space="PSUM") as pmisc,
            tc.tile_pool(name="pmp", bufs=3, space="PSUM") as pmp,
            tc.tile_pool(name="pkp", bufs=2, space="PSUM") as pkp,
            tc.tile_pool(name="dramp", bufs=1, space="DRAM") as dramp,
        ):
            # ---------------- constants / small input rows ----------------
            ident = constp.tile([128, 128], f32, name="ident")
            make_identity(nc, ident[:])
            ones_row = constp.tile([1, 128], f32, name="ones_row")
            nc.gpsimd.memset(ones_row[:], 1.0)
            ones_col = constp.tile([128, 1], f32, name="ones_col")
            nc.gpsimd.memset(ones_col[:], 1.0)
            eps_sb = constp.tile([1, 1], f32, name="eps_sb")
            nc.gpsimd.memset(eps_sb[:], EPS)

            def _row(name, src, width):
                t = sm.tile([1, width], f32, name=name)
                nc.sync.dma_start(t[:], src)
                return t

            bev_sb = _row("bev_sb", bev_r[:], L * DZ)
            lng_sb = _row("lng_sb", lng_r[:], L * DZ)
            lnb_sb = _row("lnb_sb", lnb_r[:], L * DZ)
            wg_sb = _row("wg_sb", wg_r[:], L * DZ)
            bg_sb = _row("bg_sb", bg_r[:], L)
            bval_sb = _row("bval_sb", bval_r[:], L * DLVL)
            bkey_sb = _row("bkey_sb", bkey_r[:], L * DK)
            dec_sb = _row("dec_sb", dec_r[:], L)

            # keep = 1 - decay, broadcast to all 128 partitions
            keepr = sm.tile([1, L], f32, name="keepr")
            nc.scalar.activation(keepr[:], dec_sb[:], AF.Identity,
                                 bias=1.0, scale=-1.0)
            pkeep = pmisc.tile([128, L], f32, name="pkeep", tag="pmisc")
            nc.tensor.matmul(pkeep[:], lhsT=ones_row[:], rhs=keepr[:],
                             start=True, stop=True)
            keep_bc = sm.tile([128, L], f32, name="keep_bc")
            nc.vector.tensor_copy(keep_bc[:], pkeep[:])

            # persistent per-level results (all on partition 0 rows)
            vrow = sm.tile([1, L * DLVL], f32, name="vrow")
            krow = sm.tile([1, L * DK], f32, name="krow")
            # bf16 copies feed the phase-2 rank-1 matmuls (4x PE rate);
            # the update term is ~1e-4 of the output so bf16 noise ~1e-7.
            vq = sm.tile([1, L * DLVL], bf16, name="vq")
            kq = sm.tile([1, L * DK], bf16, name="kq")
            geff = sm.tile([1, L], f32, name="geff")
            kbc = sm.tile([128, L * DK], f32, name="kbc")
            scores = sm.tile([128, L * T], f32, name="scores")
            exps = sm.tile([128, L * T], f32, name="exps")
            zpart = sm.tile([128, L], f32, name="zpart")

            ecr = dramp.tile([L * S], bf16, name="ecr")
            zcol_d = dramp.tile([L, DZ], f32, name="zcol_d")
            cc_ins = [dramp.tile([1, 8], f32, name=f"cc_in{l}")
                      for l in range(L)]
            cc_outs = [dramp.tile([1, 8], f32, name=f"cc_out{l}",
                                  addr_space="Shared") for l in range(L)]

            # ---------------- phase 0: tiny MLP (emitted per level, fused
            # into the per-level score/collective pipeline below) ----------
            def mlp_level(l):
                zsl = slice(l * DZ, (l + 1) * DZ)

                xc = wp.tile([128, 14], f32, name="xc")
                nc.sync.dma_start(xc[:], xcatT[l])
                wev = wp.tile([128, 14, 128], f32, name="wev")
                nc.sync.dma_start(
                    wev[:], wevT[l].rearrange("(c p) j -> p c j", p=128))

                pz = pmisc.tile([1, 128], f32, name="pz", tag="pmisc")
                for c in range(14):
                    nc.tensor.matmul(pz[:], lhsT=xc[:, c:c + 1],
                                     rhs=wev[:, c, :],
                                     start=(c == 0), stop=(c == 13))

                zr = zrp.tile([1, 128], f32, name="zr")
                nc.vector.tensor_tensor(zr[:], pz[:], bev_sb[:, zsl], op=A.add)

                # layernorm over the 128 free elems (single partition row)
                musum = zrp.tile([1, 1], f32, name="musum")
                nc.vector.tensor_reduce(musum[:], zr[:], axis=AX.X, op=A.add)
                mu = zrp.tile([1, 1], f32, name="mu")
                nc.scalar.mul(mu[:], musum[:], 1.0 / DZ)
                zm = zrp.tile([1, 128], f32, name="zm")
                nc.vector.tensor_scalar(zm[:], zr[:], mu[:], None, A.subtract)
                jr = junkp.tile([1, 128], f32, name="jr")
                nc.vector.tensor_tensor(jr[:], zm[:], zm[:], op=A.mult)
                vsum = zrp.tile([1, 1], f32, name="vsum")
                nc.vector.tensor_reduce(vsum[:], jr[:], axis=AX.X, op=A.add)
                std = zrp.tile([1, 1], f32, name="std")
                nc.scalar.activation(std[:], vsum[:], AF.Sqrt, bias=eps_sb[:],
                                     scale=1.0 / DZ)
                rstd = zrp.tile([1, 1], f32, name="rstd")
                nc.vector.reciprocal(rstd[:], std[:])

                zs2 = zrp.tile([1, 128], f32, name="zs2")
                nc.vector.scalar_tensor_tensor(
                    out=zs2[:], in0=zm[:], scalar=rstd[:],
                    in1=lng_sb[:, zsl], op0=A.mult, op1=A.mult)
                zs3 = zrp.tile([1, 128], f32, name="zs3")
                nc.vector.tensor_tensor(zs3[:], zs2[:], lnb_sb[:, zsl], op=A.add)
                zrow = zrp.tile([1, 128], f32, name="zrow")
                nc.vector.tensor_scalar(zrow[:], zs3[:], 0.0, None, A.max)

                # gate with sparse-write threshold
                jg = junkp.tile([1, 128], f32, name="jg")
                nc.vector.tensor_tensor(jg[:], zrow[:], wg_sb[:, zsl], op=A.mult)
                gd = zrp.tile([1, 1], f32, name="gd")
                nc.vector.tensor_reduce(gd[:], jg[:], axis=AX.X, op=A.add)
                gsig = zrp.tile([1, 1], f32, name="gsig")
                nc.scalar.activation(gsig[:], gd[:], AF.Sigmoid,
                                     bias=bg_sb[:, l:l + 1], scale=1.0)
                msk = zrp.tile([1, 1], f32, name="msk")
                nc.vector.tensor_scalar(msk[:], gsig[:], THRESH, None, A.is_ge)
                nc.vector.tensor_tensor(geff[:, l:l + 1], gsig[:], msk[:],
                                        op=A.mult)

                # z as a column for the v/k matmuls (bounce through DRAM)
                nc.sync.dma_start(
                    zcol_d[l:l + 1, :], zrow[:])
                zcol = zrp.tile([128, 1], f32, name="zcol")
                nc.sync.dma_start(
                    zcol[:], zcol_d[l].rearrange("(p a) -> p a", a=1))

                wval = wp.tile([128, DLVL], f32, name="wval")
                nc.sync.dma_start(wval[:], wvalT[l])
                pv = pmisc.tile([1, DLVL], f32, name="pv", tag="pmisc")
                nc.tensor.matmul(pv[:], lhsT=zcol[:], rhs=wval[:],
                                 start=True, stop=True)
                vsl = slice(l * DLVL, (l + 1) * DLVL)
                vpre = zrp.tile([1, DLVL], f32, name="vpre")
                nc.vector.tensor_tensor(vpre[:], pv[:], bval_sb[:, vsl], op=A.add)
                nc.scalar.activation(vrow[:, vsl], vpre[:], AF.Tanh)

                wkey = wp.tile([128, DK], f32, name="wkey")
                nc.sync.dma_start(wkey[:], wkeyT[l])
                pk0 = pmisc.tile([1, DK], f32, name="pk0", tag="pmisc")
                nc.tensor.matmul(pk0[:], lhsT=zcol[:], rhs=wkey[:],
                                 start=True, stop=True)
                ksl = slice(l * DK, (l + 1) * DK)
                nc.vector.tensor_tensor(krow[:, ksl], pk0[:], bkey_sb[:, ksl],
                                        op=A.add)

                # broadcast k/sqrt(DK) across all partitions (rank-1 with ones)
                ksc = zrp.tile([1, DK], f32, name="ksc")
                nc.vector.tensor_scalar(ksc[:], krow[:, ksl], SCALE, None,
                                        A.mult)
                pkb = pmisc.tile([128, DK], f32, name="pkb", tag="pmisc")
                nc.tensor.matmul(pkb[:], lhsT=ones_row[:], rhs=ksc[:],
                                 start=True, stop=True)
                nc.vector.tensor_copy(kbc[:, ksl], pkb[:])

            # ---- phase 1 (per level): stream K, scores, exp, ecr bounce,
            # per-level AllReduce of the softmax normalizer, v'/k' scaling.
            # Level l's phase-2 streaming unblocks as soon as ITS collective
            # lands, overlapping with later levels' score passes.
            inv = sm.tile([1, L], f32, name="inv")
            for l in range(L):
                mlp_level(l)
                ksl = slice(l * DK, (l + 1) * DK)
                vsl = slice(l * DLVL, (l + 1) * DLVL)
                for ch in range(NCH):
                    ki = strm.tile([128, SUB, DK], f32, name="ki", bufs=3)
                    nc.scalar.dma_start(
                        ki[:],
                        Kp[l, ch * CH_SL:(ch + 1) * CH_SL, :]
                        .rearrange("(p t) d -> p t d", t=SUB))
                    for t in range(SUB):
                        lt = l * T + ch * SUB + t
                        jk = junkp.tile([128, 128], f32, name="jk")
                        nc.vector.scalar_tensor_tensor(
                            out=jk[:], in0=ki[:, t, :], scalar=1.0,
                            in1=kbc[:, ksl], op0=A.mult, op1=A.mult,
                            accum_out=scores[:, lt:lt + 1])
                nc.scalar.activation(exps[:, l * T:(l + 1) * T],
                                     scores[:, l * T:(l + 1) * T], AF.Exp)
                nc.vector.tensor_reduce(zpart[:, l:l + 1],
                                        exps[:, l * T:(l + 1) * T],
                                        axis=AX.X, op=A.add)

                pt = pmisc.tile([64, 128], f32, name="pt", tag="pmisc")
                nc.tensor.transpose(pt[:], exps[:, l * T:(l + 1) * T], ident[:])
                et = zrp.tile([64, 128], bf16, name="et")
                nc.vector.tensor_copy(et[:], pt[:])
                nc.sync.dma_start(
                    ecr[l * S:(l + 1) * S].rearrange("(t s) -> t s", s=128),
                    et[:])

                pz1 = pmisc.tile([1, 1], f32, name="pz1", tag="pmisc")
                nc.tensor.matmul(pz1[:], lhsT=ones_col[:],
                                 rhs=zpart[:, l:l + 1], start=True, stop=True)
                z1 = zrp.tile([1, 8], f32, name="z1")
                nc.gpsimd.memset(z1[:], 0.0)
                nc.vector.tensor_copy(z1[:, 0:1], pz1[:])

                nc.gpsimd.dma_start(cc_ins[l][:], z1[:])
                if not SKIP_CC:
                    nc.gpsimd.collective_compute(
                        "AllReduce", A.add,
                        replica_groups=[list(range(NCORES))],
                        ins=[cc_ins[l].opt()], outs=[cc_outs[l].opt()])
                else:
                    nc.gpsimd.dma_start(cc_outs[l][:], cc_ins[l][:])
                zg = zrp.tile([1, 8], f32, name="zg")
                nc.gpsimd.dma_start(zg[:], cc_outs[l][:])

                zrcp = zrp.tile([1, 1], f32, name="zrcp")
                nc.vector.reciprocal(zrcp[:], zg[:, 0:1])
                nc.vector.tensor_tensor(inv[:, l:l + 1], geff[:, l:l + 1],
                                        zrcp[:], op=A.mult)
                nc.vector.tensor_scalar(vq[:, vsl], vrow[:, vsl],
                                        inv[:, l:l + 1], None, A.mult)
                nc.vector.tensor_scalar(kq[:, ksl], krow[:, ksl],
                                        inv[:, l:l + 1], None, A.mult)

            # ---------------- phase 2: stream M / K updates ----------------
            for l in range(L):
                vsl = slice(l * DLVL, (l + 1) * DLVL)
                ksl = slice(l * DK, (l + 1) * DK)
                keep_sc = keep_bc[:, l:l + 1]
                for c in range(NCH):
                    base = c * CH_SL
                    ei = strm.tile([1, CH_SL], bf16, name="ei", bufs=3)
                    nc.gpsimd.dma_start(
                        ei[:],
                        ecr[l * S + base:l * S + base + CH_SL].rearrange(
                            "(a x) -> a x", a=1))
                    mk = strm.tile([128, SUB, DLVL + DK], f32, name="mk",
                                   bufs=3)
                    nc.sync.dma_start(
                        mk[:],
                        MKp[l, base:base + CH_SL, :].rearrange(
                            "(p t) d -> p t d", t=SUB))
                    mo = strm.tile([128, SUB, DLVL + DK], f32, name="mo",
                                   bufs=3)
                    for t in range(SUB):
                        et_sl = ei[:, t * 128:(t + 1) * 128]
                        pm = pmp.tile([128, DLVL], f32, name="pm", tag="pm")
                        nc.tensor.matmul(pm[:], lhsT=et_sl,
                                         rhs=vq[:, vsl], start=True, stop=True)
                        nc.vector.scalar_tensor_tensor(
                            out=mo[:, t, 0:DLVL], in0=mk[:, t, 0:DLVL],
                            scalar=keep_sc, in1=pm[:], op0=A.mult, op1=A.add)
                        pkk = pkp.tile([128, DK], f32, name="pkk", tag="pk")
                        nc.tensor.matmul(pkk[:], lhsT=et_sl,
                                         rhs=kq[:, ksl], start=True, stop=True)
                        nc.vector.scalar_tensor_tensor(
                            out=mo[:, t, DLVL:DLVL + DK],
                            in0=mk[:, t, DLVL:DLVL + DK],
                            scalar=keep_sc, in1=pkk[:], op0=A.mult, op1=A.add)
                    nc.scalar.dma_start(
                        out[l, base:base + CH_SL, :].rearrange(
                            "(p t) d -> p t d", t=SUB), mo[:])

    nc.compile()
    return nc


def _prep_in_maps(inputs):
    f32 = np.float32
    s_t = np.asarray(inputs["s_t"], f32)
    e_t = np.asarray(inputs["e_t"], f32)
    lc = np.asarray(inputs["level_contexts"], f32)
    W_ev0 = np.asarray(inputs["W_ev0"], f32)
    W_ev = np.asarray(inputs["W_ev"], f32)
    b_ev = np.asarray(inputs["b_ev"], f32)
    ln_g = np.asarray(inputs["ln_g"], f32)
    ln_b = np.asarray(inputs["ln_b"], f32)
    W_gate = np.asarray(inputs["W_gate"], f32)
    b_gate = np.asarray(inputs["b_gate"], f32)
    W_val = np.asarray(inputs["W_val"], f32)
    b_val = np.asarray(inputs["b_val"], f32)
    W_key = np.asarray(inputs["W_key"], f32)
    b_key = np.asarray(inputs["b_key"], f32)
    M = np.asarray(inputs["M"], f32)
    K = np.asarray(inputs["K"], f32)
    decay = np.asarray(inputs["decay"], f32)

    # unified MLP input per level: level 0 uses [s, e, 0-pad], levels 1-2 use
    # [s, ctx, e]; weight matrices padded/stacked to match.
    xcat = np.zeros((L, 1792), f32)
    xcat[0, :1024] = s_t
    xcat[0, 1024:1536] = e_t
    for l in (1, 2):
        xcat[l] = np.concatenate([s_t, lc[l - 1], e_t])
    xcatT = np.ascontiguousarray(
        xcat.reshape(L, 14, 128).transpose(0, 2, 1))
    W0p = np.concatenate([W_ev0, np.zeros((DZ, 256), f32)], axis=1)
    Wfull = np.stack([W0p, W_ev[0], W_ev[1]])
    wevT = np.ascontiguousarray(Wfull.transpose(0, 2, 1))
    wvalT = np.ascontiguousarray(W_val.transpose(0, 2, 1))
    wkeyT = np.ascontiguousarray(W_key.transpose(0, 2, 1))

    shared = dict(
        xcatT=xcatT, wevT=wevT, wvalT=wvalT, wkeyT=wkeyT,
        bev_r=b_ev.reshape(1, -1), lng_r=ln_g.reshape(1, -1),
        lnb_r=ln_b.reshape(1, -1), wg_r=W_gate.reshape(1, -1),
        bg_r=b_gate.reshape(1, -1), bval_r=b_val.reshape(1, -1),
        bkey_r=b_key.reshape(1, -1), dec_r=decay.reshape(1, -1),
    )
    MK = np.concatenate([M, K], axis=2)
    in_maps = []
    for c in range(NCORES):
        sl = slice(c * S, (c + 1) * S)
        m = dict(shared)
        m["MKp"] = np.ascontiguousarray(MK[:, sl, :])
        m["Kp"] = np.ascontiguousarray(K[:, sl, :])
        in_maps.append(m)
    return in_maps


def _run(inputs, trace=False):
    import concourse.bass_utils as bass_utils

    nc = _STATE.get("nc")
    if nc is None:
        nc = _build_bass()
        _STATE["nc"] = nc
    in_maps = _prep_in_maps(inputs)
    res = bass_utils.run_bass_kernel_spmd(
        nc, in_maps, core_ids=list(range(NCORES)), trace=trace)
    full = np.concatenate([res.results[c]["out"] for c in range(NCORES)],
                          axis=1)
    return full.astype(np.float32, copy=False), res


def kernel(**inputs):
    out, _ = _run(inputs, trace=False)
    return out

